# revision 24
# baseline (speedup 1.0000x reference)
"""Trainium2 Bass kernel for windowed (inverted-window) attention.

Problem: B=2, T=2048, C=2048, H=16 heads, D=128, WINDOW=512.
  q,k,v = x@Wq, x@Wk, x@Wv  (per-head reshape), RoPE on q,k,
  scores masked so positions INSIDE the causal window are masked out
  (attend only to j>i or j<i-511), softmax, o@Wo.

Sharding: 8 cores = 2 (batch) x 4 (head groups of 4 heads).
Each core computes its batch's 4 heads end-to-end plus a partial
output projection (row-chunk of Wo); host sums the 4 partials per batch.

Matmul operands are bf16 (fp32 PSUM accumulation); everything else fp32.

Optimizations over the first working version:
 - single PSUM pool (4 tags x 2 banks) spanning both phases: no pool-close
   barrier between projections and attention.
 - phase A grouped as K-sweep / Q-sweep / V-sweep per t-block with PSUM
   bank alternation, so RoPE drains overlap the next sweep's matmuls.
 - score/AV matmuls restricted to the contiguous allowed q-range per
   (i-block, k-chunk): fully-masked 128-wide subtiles are never computed.
   The mask is applied as min(et, {0,1e30}) which also zeroes stale
   columns, keeping the softmax-denominator tree exact.
 - RoPE arithmetic in bf16 (DVE 2x mode), exp on union ranges only.
 - PSUM->SBUF output copies on the (otherwise idle) Pool engine.
 - Wo kept resident in SBUF; mask/Wo loads issued during phase A.
"""

import sys
import numpy as np

for _p in ("/opt/trn_rl_repo",):
    if _p not in sys.path:
        sys.path.insert(0, _p)

import ml_dtypes  # noqa: E402

# If BASS_TRACE is set in the environment, run_bass_kernel_spmd imports
# antenv.axon_hooks, which this container does not ship. Register a stub
# so tracing degrades gracefully instead of crashing.
try:
    import antenv.axon_hooks  # noqa: F401
except ImportError:
    import types as _types

    _hooks = _types.ModuleType("antenv.axon_hooks")
    _hooks._hook = None
    _hooks.set_axon_ntff_profile_hook = lambda h: setattr(_hooks, "_hook", h)
    _hooks.get_axon_ntff_profile_hook = lambda: _hooks._hook
    sys.modules["antenv.axon_hooks"] = _hooks
    import antenv as _antenv

    _antenv.axon_hooks = _hooks
import concourse.bass as bass  # noqa: E402
import concourse.mybir as mybir  # noqa: E402
from concourse.bacc import Bacc  # noqa: E402
from concourse.tile import TileContext  # noqa: E402
from concourse.bass import ts, ds  # noqa: E402
from concourse.bass_utils import run_bass_kernel_spmd  # noqa: E402

B, T, C, H, D = 2, 2048, 2048, 16, 128
HL = 4                # heads per core
NCORES = 8
WINDOW = 512
ROPE_BASE = 10000.0
TB = 512              # i/t block size (matmul free dim)
NTB = T // TB         # 4
CK = C // 128         # 16 contraction chunks for projections
NTC = T // 128        # 16 j-chunks / t-chunks
MASK_W = 256          # two 128x128 partial-subtile masks (delta = 0 and 4)
F32 = mybir.dt.float32
BF16 = mybir.dt.bfloat16
AF = mybir.ActivationFunctionType

MM_DT = BF16          # dtype of every matmul operand tensor
NP_MM = ml_dtypes.bfloat16

_NC = None
TRACE = False
LAST_RESULT = None    # BassKernelResults of the most recent run (for test.py)


def _q_range(ib, c):
    """Contiguous allowed q-subchunk range for i-block ib vs key chunk c.

    Subtile (qsub s, key chunk c) is fully inside the masked window iff
    1 <= s - c <= 3 (then every i-j lies in [0, 511]).  Returns (lo, hi)
    in elements within the 512-wide i-block.
    """
    subs = [s for s in range(4 * ib, 4 * ib + 4) if not (1 <= s - c <= 3)]
    lo = (min(subs) - 4 * ib) * 128
    hi = (max(subs) - 4 * ib) * 128 + 128
    return lo, hi


def build_nc():
    nc = Bacc()
    xT = nc.declare_dram_parameter("xT", [C, T], MM_DT, isOutput=False)
    wq = nc.declare_dram_parameter("wq", [C, HL * D], MM_DT, isOutput=False)
    wk = nc.declare_dram_parameter("wk", [C, HL * D], MM_DT, isOutput=False)
    wv = nc.declare_dram_parameter("wv", [C, HL * D], MM_DT, isOutput=False)
    wo = nc.declare_dram_parameter("wo", [HL * D, C], MM_DT, isOutput=False)
    cosx = nc.declare_dram_parameter("cosx", [128, T], MM_DT, isOutput=False)
    sinx = nc.declare_dram_parameter("sinx", [128, T], MM_DT, isOutput=False)
    maskm = nc.declare_dram_parameter("maskm", [128, MASK_W], MM_DT, isOutput=False)
    out = nc.declare_dram_parameter("out", [T, C], F32, isOutput=True)

    xT_v = xT[:].rearrange("(co p) t -> p co t", p=128)   # [128, 16, T]
    wq_v = wq[:].rearrange("(co p) d -> p co d", p=128)   # [128, 16, 512]
    wk_v = wk[:].rearrange("(co p) d -> p co d", p=128)
    wv_v = wv[:].rearrange("(co p) d -> p co d", p=128)
    wo_v = wo[:].rearrange("(h p) c -> p h c", p=128)     # [128, 4, C]

    scale = float(1.0 / np.sqrt(D))

    with TileContext(nc) as tc:
        with (
            tc.tile_pool(name="res", bufs=1) as res,      # long-lived residents
            tc.tile_pool(name="xbp", bufs=16) as xbp,     # streamed x chunks
            tc.tile_pool(name="ropet", bufs=4) as ropet,
            tc.tile_pool(name="ropes", bufs=4) as ropes,
            tc.tile_pool(name="etp", bufs=10) as etp,
            tc.tile_pool(name="smp", bufs=2) as smp,
            tc.tile_pool(name="zp", bufs=1) as zp,
            tc.tile_pool(name="ocb", bufs=4) as ocb,
            tc.tile_pool(name="psum", bufs=1, space="PSUM") as psum,
        ):
            # ---- long-lived tensors ----
            wqs, wks = [], []
            for ck in range(CK):
                wqc = res.tile([128, HL * D], MM_DT, tag=f"wq{ck}", name=f"wq{ck}")
                nc.sync.dma_start(wqc[:], wq_v[:, ck, :])
                wkc = res.tile([128, HL * D], MM_DT, tag=f"wk{ck}", name=f"wk{ck}")
                nc.sync.dma_start(wkc[:], wk_v[:, ck, :])
                wqs.append(wqc)
                wks.append(wkc)
            wvt = res.tile([128, CK, HL * D], MM_DT)
            nc.sync.dma_start(wvt[:], wv_v[:])
            cosb = res.tile([128, T], MM_DT)
            nc.sync.dma_start(cosb[:], cosx[:])
            sinb = res.tile([128, T], MM_DT)
            nc.sync.dma_start(sinb[:], sinx[:])
            maskb = res.tile([128, MASK_W], MM_DT)
            nc.sync.dma_start(maskb[:], maskm[:])
            wof = res.tile([128, HL, C], MM_DT)           # resident Wo
            nc.sync.dma_start(wof[:], wo_v[:])

            QT = res.tile([128, HL, T], MM_DT)    # q transposed [d, t]
            KT = res.tile([128, HL, T], MM_DT)
            V = res.tile([128, NTC, HL * D], MM_DT)   # v natural [t, hd]
            oT = res.tile([128, HL, T], MM_DT)    # per-head o transposed [d, t]
            ones = res.tile([128, 128], MM_DT)
            nc.vector.memset(ones[:], 1.0)

            # PSUM: 4 persistent double-bank tiles, alternated between phases
            def pstile(tag, nm):
                return psum.tile([128, 2, TB], F32, tag=tag, name=nm)

            def rope(ps_slice, OUTT, h, tb):
                # RoPE: out = raw*cos + swap(raw)*sin_signed, all bf16
                raw = ropet.tile([128, TB], MM_DT, tag="raw")
                nc.scalar.copy(raw[:], ps_slice)
                sw = ropes.tile([128, TB], MM_DT, tag="sw")
                # issue the half-swap DMAs from the SWDGE (gpsimd) queue: the
                # sync queue is busy streaming weights during phase A
                nc.gpsimd.dma_start(sw[0:64, :], raw[64:128, :])
                nc.gpsimd.dma_start(sw[64:128, :], raw[0:64, :])
                nc.vector.tensor_mul(sw[:], sw[:], sinb[:, ts(tb, TB)])
                nc.vector.tensor_mul(raw[:], raw[:], cosb[:, ts(tb, TB)])
                nc.vector.tensor_add(OUTT[:, h, ts(tb, TB)], sw[:], raw[:])

            # ---- Phase A: projections (QK transposed + RoPE, V natural) ----
            for tb in range(NTB):
                xbs = []
                for ck in range(CK):
                    xb = xbp.tile([128, TB], MM_DT, tag="xtb", name=f"xb{tb}_{ck}")
                    nc.gpsimd.dma_start(xb[:], xT_v[:, ck, ts(tb, TB)])
                    xbs.append(xb)
                # bank assignment alternates per tb so sweeps pipeline
                a, b = (0, 2) if tb % 2 == 0 else (2, 0)
                pK = [pstile(f"P{a}", f"k{tb}a"), pstile(f"P{a+1}", f"k{tb}b")]
                pQ = [pstile(f"P{b}", f"q{tb}a"), pstile(f"P{b+1}", f"q{tb}b")]
                # K sweep
                for ck in range(CK):
                    for h in range(HL):
                        nc.tensor.matmul(
                            pK[h // 2][:, h % 2, :], wks[ck][:, ts(h, D)], xbs[ck][:],
                            start=(ck == 0), stop=(ck == CK - 1),
                        )
                for h in range(HL):
                    rope(pK[h // 2][:, h % 2, :], KT, h, tb)
                # Q sweep
                for ck in range(CK):
                    for h in range(HL):
                        nc.tensor.matmul(
                            pQ[h // 2][:, h % 2, :], wqs[ck][:, ts(h, D)], xbs[ck][:],
                            start=(ck == 0), stop=(ck == CK - 1),
                        )
                for h in range(HL):
                    rope(pQ[h // 2][:, h % 2, :], QT, h, tb)
                # V sweep (reuses K banks, freed by the K RoPE copies)
                pV = [pstile(f"P{a}", f"v{tb}a"), pstile(f"P{a+1}", f"v{tb}b")]
                for tco in range(NTB):
                    pv = pV[tco // 2][:, tco % 2, :]
                    for ck in range(CK):
                        nc.tensor.matmul(
                            pv, xbs[ck][:, ts(tco, 128)], wvt[:, ck, :],
                            start=(ck == 0), stop=(ck == CK - 1),
                        )
                    nc.scalar.copy(V[:, tb * NTB + tco, :], pv)

            # ---- Phase B: attention + interleaved output projection ----
            # The softmax denominator is computed on the PE (ones-matmuls
            # accumulating over the same restricted ranges as the A@V pass),
            # so no elementwise reduction tree is needed. Junk columns of et
            # (outside a chunk's allowed q-range) are never read; only the
            # partial diagonal subtiles need an explicit {0,1} mask multiply.
            for ib in range(NTB):
                ranges = [_q_range(ib, c) for c in range(NTC)]
                full = [c for c in range(NTC) if ranges[c] == (0, TB)]
                ff, fl = full[0], full[-1]
                order = [ff] + [c for c in range(NTC) if c not in (ff, fl)] + [fl]
                for h in range(HL):
                    ets = []
                    for cp in range(NTC // 2):
                        ps = pstile(f"P{cp % 2}", f"s{h}_{ib}_{cp}")
                        for k in range(2):
                            c = 2 * cp + k
                            lo, hi = ranges[c]
                            nc.tensor.matmul(
                                ps[:, k, ds(lo, hi - lo)],
                                KT[:, h, ts(c, 128)],
                                QT[:, h, ds(ib * TB + lo, hi - lo)],
                                start=True, stop=True,
                            )
                        et = etp.tile([128, 2, TB], MM_DT, tag="et")
                        # exp only the union of the two chunks' q-ranges:
                        # nothing ever reads et outside the per-chunk ranges
                        ulo = min(ranges[2 * cp][0], ranges[2 * cp + 1][0])
                        uhi = max(ranges[2 * cp][1], ranges[2 * cp + 1][1])
                        nc.scalar.activation(
                            et[:, :, ds(ulo, uhi - ulo)],
                            ps[:, :, ds(ulo, uhi - ulo)], AF.Exp, scale=scale,
                        )
                        # zero the (at most one) partial diagonal subtile
                        for k in range(2):
                            c = 2 * cp + k
                            for s in range(4 * ib, 4 * ib + 4):
                                if s - c in (0, 4):
                                    so = (s - 4 * ib) * 128
                                    mo = 0 if s - c == 0 else 128
                                    nc.vector.tensor_mul(
                                        et[:, k, ds(so, 128)],
                                        et[:, k, ds(so, 128)],
                                        maskb[:, ds(mo, 128)],
                                    )
                        ets.append(et)
                    # pre-sum pairs of full-range chunks on DVE so the PE
                    # denominator matmuls only cover the reduced set
                    fullpairs = [
                        cp for cp in range(NTC // 2)
                        if ranges[2 * cp] == (0, TB) and ranges[2 * cp + 1] == (0, TB)
                    ]
                    rest = [c for c in range(NTC) if c // 2 not in fullpairs]
                    us = []
                    for j, cp in enumerate(fullpairs):
                        u = zp.tile([128, TB], MM_DT, tag=f"u{j}", name=f"u{h}_{ib}_{j}")
                        nc.vector.tensor_add(u[:], ets[cp][:, 0, :], ets[cp][:, 1, :])
                        us.append(u)
                    qs = []
                    for j in range(0, len(us) - 1, 2):
                        nc.vector.tensor_add(us[j][:], us[j][:], us[j + 1][:])
                        qs.append(us[j])
                    if len(us) % 2:
                        qs.append(us[-1])
                    poz = pstile("P2", f"poz{h}_{ib}")
                    pso = poz[:, 0, :]
                    psz = poz[:, 1, :]
                    for idx, c in enumerate(order):
                        lo, hi = ranges[c]
                        nc.tensor.matmul(
                            pso[:, ds(lo, hi - lo)], V[:, c, ts(h, D)],
                            ets[c // 2][:, c % 2, ds(lo, hi - lo)],
                            start=(idx == 0), stop=(idx == NTC - 1),
                        )
                    # denominator: qs tiles (full width) + leftover chunks;
                    # first and last summands are full-width for start/stop
                    rest_f = [c for c in rest if ranges[c] == (0, TB)]
                    rest_p = [c for c in rest if ranges[c] != (0, TB)]
                    nz = len(qs) + len(rest) - 1
                    i = 0
                    nc.tensor.matmul(
                        psz, ones[:], qs[0][:], start=True, stop=(nz == 0)
                    )
                    for q in qs[1:]:
                        i += 1
                        nc.tensor.matmul(
                            psz, ones[:], q[:], start=False, stop=(i == nz)
                        )
                    for c in rest_p + rest_f:
                        i += 1
                        lo, hi = ranges[c]
                        nc.tensor.matmul(
                            psz[:, ds(lo, hi - lo)], ones[:],
                            ets[c // 2][:, c % 2, ds(lo, hi - lo)],
                            start=False, stop=(i == nz),
                        )
                    rz = smp.tile([128, TB], F32, tag="rz")
                    nc.vector.reciprocal_approx_fast(rz[:], psz)
                    nc.vector.tensor_mul(oT[:, h, ts(ib, TB)], pso, rz[:])
                # output projection for this i-block (all 4 heads done).
                # 1024-wide matmuls fill a whole 2-bank PSUM tile; alternate
                # tags P3/P2 so the Pool drain of one overlaps the next.
                g = 0
                for tto in range(NTB):
                    tt = ib * NTB + tto
                    for cbp in range(2):
                        po = pstile(f"P{3 - g % 2}", f"po{ib}_{tto}_{cbp}")
                        g += 1
                        for half in range(2):
                            for h in range(HL):
                                nc.tensor.matmul(
                                    po[:, half, :], oT[:, h, ts(tt, 128)],
                                    wof[:, h, ds((2 * cbp + half) * TB, TB)],
                                    start=(h == 0), stop=(h == HL - 1),
                                )
                        ob = ocb.tile([128, 2, TB], F32, tag="ob")
                        nc.vector.tensor_copy(ob[:], po[:])
                        nc.sync.dma_start(
                            out[ts(tt, 128), ds(cbp * 2 * TB, 2 * TB)], ob[:]
                        )

    nc.finalize()
    return nc


def _host_tables():
    inv_freq = (
        1.0 / (np.float32(ROPE_BASE) ** (np.arange(0, D, 2, dtype=np.float32) / np.float32(D)))
    ).astype(np.float32)
    t = np.arange(T, dtype=np.float32)
    freqs = (t[:, None] * inv_freq[None, :]).astype(np.float32)  # [T, 64]
    cos = np.cos(freqs).T.astype(np.float32)                     # [64, T]
    sin = np.sin(freqs).T.astype(np.float32)
    cosx = np.ascontiguousarray(np.concatenate([cos, cos], axis=0)).astype(NP_MM)
    sinx = np.ascontiguousarray(np.concatenate([-sin, sin], axis=0)).astype(NP_MM)
    p = np.arange(128, dtype=np.int64)[:, None]
    f = np.arange(128, dtype=np.int64)[None, :]
    blocks = []
    for delta_chunk in (0, 4):
        delta = 128 * delta_chunk + f - p          # i - j within the subtile
        blocks.append(~((delta >= 0) & (delta <= WINDOW - 1)))
    maskm = np.ascontiguousarray(np.concatenate(blocks, axis=1).astype(NP_MM))
    return cosx, sinx, maskm


def kernel(x, Wq, Wk, Wv, Wo):
    global _NC, LAST_RESULT
    if _NC is None:
        _NC = build_nc()
    x = np.asarray(x, dtype=np.float32)
    Wq = np.asarray(Wq, dtype=np.float32)
    Wk = np.asarray(Wk, dtype=np.float32)
    Wv = np.asarray(Wv, dtype=np.float32)
    Wo = np.asarray(Wo, dtype=np.float32)
    cosx, sinx, maskm = _host_tables()
    in_maps = []
    for core in range(NCORES):
        b, hg = divmod(core, NCORES // B)
        sl = slice(hg * HL * D, (hg + 1) * HL * D)
        in_maps.append(
            {
                "xT": np.ascontiguousarray(x[b].T.astype(NP_MM)),
                "wq": np.ascontiguousarray(Wq[:, sl].astype(NP_MM)),
                "wk": np.ascontiguousarray(Wk[:, sl].astype(NP_MM)),
                "wv": np.ascontiguousarray(Wv[:, sl].astype(NP_MM)),
                "wo": np.ascontiguousarray(Wo[sl, :].astype(NP_MM)),
                "cosx": cosx,
                "sinx": sinx,
                "maskm": maskm,
            }
        )
    res = run_bass_kernel_spmd(_NC, in_maps, list(range(NCORES)), trace=TRACE)
    LAST_RESULT = res
    out = np.zeros((B, T, C), dtype=np.float32)
    for core in range(NCORES):
        b = core // (NCORES // B)
        out[b] += res.results[core]["out"]
    return out


# revision 25
# speedup vs baseline: 1.1565x; 1.1565x over previous
"""Trainium2 Bass kernel for windowed (inverted-window) attention.

Problem: B=2, T=2048, C=2048, H=16 heads, D=128, WINDOW=512.
  q,k,v = x@Wq, x@Wk, x@Wv  (per-head reshape), RoPE on q,k,
  scores masked so positions INSIDE the causal window are masked out
  (attend only to j>i or j<i-511), softmax, o@Wo.

Sharding: 8 cores = 2 (batch) x 4 (head groups of 4 heads).
Each core computes its batch's 4 heads end-to-end plus a partial
output projection (row-chunk of Wo); host sums the 4 partials per batch.

Matmul operands are bf16 (fp32 PSUM accumulation); everything else fp32.

Optimizations over the first working version:
 - single PSUM pool (4 tags x 2 banks) spanning both phases: no pool-close
   barrier between projections and attention.
 - phase A grouped as K-sweep / Q-sweep / V-sweep per t-block with PSUM
   bank alternation, so RoPE drains overlap the next sweep's matmuls.
 - score/AV matmuls restricted to the contiguous allowed q-range per
   (i-block, k-chunk): fully-masked 128-wide subtiles are never computed.
   The mask is applied as min(et, {0,1e30}) which also zeroes stale
   columns, keeping the softmax-denominator tree exact.
 - RoPE arithmetic in bf16 (DVE 2x mode), exp on union ranges only.
 - PSUM->SBUF output copies on the (otherwise idle) Pool engine.
 - Wo kept resident in SBUF; mask/Wo loads issued during phase A.
"""

import sys
import numpy as np

for _p in ("/opt/trn_rl_repo",):
    if _p not in sys.path:
        sys.path.insert(0, _p)

import ml_dtypes  # noqa: E402

# If BASS_TRACE is set in the environment, run_bass_kernel_spmd imports
# antenv.axon_hooks, which this container does not ship. Register a stub
# so tracing degrades gracefully instead of crashing.
try:
    import antenv.axon_hooks  # noqa: F401
except ImportError:
    import types as _types

    _hooks = _types.ModuleType("antenv.axon_hooks")
    _hooks._hook = None
    _hooks.set_axon_ntff_profile_hook = lambda h: setattr(_hooks, "_hook", h)
    _hooks.get_axon_ntff_profile_hook = lambda: _hooks._hook
    sys.modules["antenv.axon_hooks"] = _hooks
    import antenv as _antenv

    _antenv.axon_hooks = _hooks
import concourse.bass as bass  # noqa: E402
import concourse.mybir as mybir  # noqa: E402
from concourse.bacc import Bacc  # noqa: E402
from concourse.tile import TileContext  # noqa: E402
from concourse.bass import ts, ds  # noqa: E402
from concourse.bass_utils import run_bass_kernel_spmd  # noqa: E402

B, T, C, H, D = 2, 2048, 2048, 16, 128
HL = 4                # heads per core
NCORES = 8
WINDOW = 512
ROPE_BASE = 10000.0
TB = 512              # i/t block size (matmul free dim)
NTB = T // TB         # 4
CK = C // 128         # 16 contraction chunks for projections
NTC = T // 128        # 16 j-chunks / t-chunks
MASK_W = 256          # two 128x128 partial-subtile masks (delta = 0 and 4)
F32 = mybir.dt.float32
BF16 = mybir.dt.bfloat16
AF = mybir.ActivationFunctionType

MM_DT = BF16          # dtype of every matmul operand tensor
NP_MM = ml_dtypes.bfloat16

_NC = None
TRACE = False
LAST_RESULT = None    # BassKernelResults of the most recent run (for test.py)


def _q_range(ib, c):
    """Contiguous allowed q-subchunk range for i-block ib vs key chunk c.

    Subtile (qsub s, key chunk c) is fully inside the masked window iff
    1 <= s - c <= 3 (then every i-j lies in [0, 511]).  Returns (lo, hi)
    in elements within the 512-wide i-block.
    """
    subs = [s for s in range(4 * ib, 4 * ib + 4) if not (1 <= s - c <= 3)]
    lo = (min(subs) - 4 * ib) * 128
    hi = (max(subs) - 4 * ib) * 128 + 128
    return lo, hi


def build_nc():
    nc = Bacc()
    xT = nc.declare_dram_parameter("xT", [C, T], MM_DT, isOutput=False)
    wq = nc.declare_dram_parameter("wq", [C, HL * D], MM_DT, isOutput=False)
    wk = nc.declare_dram_parameter("wk", [C, HL * D], MM_DT, isOutput=False)
    wv = nc.declare_dram_parameter("wv", [C, HL * D], MM_DT, isOutput=False)
    wo = nc.declare_dram_parameter("wo", [HL * D, C], MM_DT, isOutput=False)
    cosx = nc.declare_dram_parameter("cosx", [128, T], MM_DT, isOutput=False)
    sinx = nc.declare_dram_parameter("sinx", [128, T], MM_DT, isOutput=False)
    maskm = nc.declare_dram_parameter("maskm", [128, MASK_W], MM_DT, isOutput=False)
    out = nc.declare_dram_parameter("out", [T, C], F32, isOutput=True)

    xT_v = xT[:].rearrange("(co p) t -> p co t", p=128)   # [128, 16, T]
    wq_v = wq[:].rearrange("(co p) d -> p co d", p=128)   # [128, 16, 512]
    wk_v = wk[:].rearrange("(co p) d -> p co d", p=128)
    wv_v = wv[:].rearrange("(co p) d -> p co d", p=128)
    wo_v = wo[:].rearrange("(h p) c -> p h c", p=128)     # [128, 4, C]

    scale = float(1.0 / np.sqrt(D))

    with TileContext(nc) as tc:
        with (
            tc.tile_pool(name="res", bufs=1) as res,      # long-lived residents
            tc.tile_pool(name="xbp", bufs=16) as xbp,     # streamed x chunks
            tc.tile_pool(name="ropet", bufs=4) as ropet,
            tc.tile_pool(name="ropes", bufs=4) as ropes,
            tc.tile_pool(name="etp", bufs=10) as etp,
            tc.tile_pool(name="smp", bufs=2) as smp,
            tc.tile_pool(name="zp", bufs=1) as zp,
            tc.tile_pool(name="ocb", bufs=4) as ocb,
            tc.tile_pool(name="psum", bufs=1, space="PSUM") as psum,
        ):
            # ---- long-lived tensors ----
            wqs, wks = [], []
            for ck in range(CK):
                wqc = res.tile([128, HL * D], MM_DT, tag=f"wq{ck}", name=f"wq{ck}")
                nc.sync.dma_start(wqc[:], wq_v[:, ck, :])
                wkc = res.tile([128, HL * D], MM_DT, tag=f"wk{ck}", name=f"wk{ck}")
                nc.sync.dma_start(wkc[:], wk_v[:, ck, :])
                wqs.append(wqc)
                wks.append(wkc)
            wvt = res.tile([128, CK, HL * D], MM_DT)
            nc.sync.dma_start(wvt[:], wv_v[:])
            cosb = res.tile([128, T], MM_DT)
            nc.sync.dma_start(cosb[:], cosx[:])
            sinb = res.tile([128, T], MM_DT)
            nc.sync.dma_start(sinb[:], sinx[:])
            maskb = res.tile([128, MASK_W], MM_DT)
            nc.sync.dma_start(maskb[:], maskm[:])
            wof = res.tile([128, HL, C], MM_DT)           # resident Wo
            nc.sync.dma_start(wof[:], wo_v[:])

            QT = res.tile([128, HL, T], MM_DT)    # q transposed [d, t]
            KT = res.tile([128, HL, T], MM_DT)
            V = res.tile([128, NTC, HL * D], MM_DT)   # v natural [t, hd]
            oT = res.tile([128, HL, T], MM_DT)    # per-head o transposed [d, t]
            ones = res.tile([128, 128], MM_DT)
            nc.vector.memset(ones[:], 1.0)

            # PSUM: 4 persistent double-bank tiles, alternated between phases
            def pstile(tag, nm):
                return psum.tile([128, 2, TB], F32, tag=tag, name=nm)

            def rope(ps_slice, OUTT, h, tb):
                # RoPE: out = raw*cos + swap(raw)*sin_signed, all bf16
                raw = ropet.tile([128, TB], MM_DT, tag="raw")
                nc.scalar.copy(raw[:], ps_slice)
                sw = ropes.tile([128, TB], MM_DT, tag="sw")
                # issue the half-swap DMAs from the scalar queue's DGE: the
                # sync queue streams weights and the SWDGE queue must stay
                # free for x prefetch (its FIFO is strictly in-order)
                nc.scalar.dma_start(sw[0:64, :], raw[64:128, :])
                nc.scalar.dma_start(sw[64:128, :], raw[0:64, :])
                nc.vector.tensor_mul(sw[:], sw[:], sinb[:, ts(tb, TB)])
                nc.vector.tensor_mul(raw[:], raw[:], cosb[:, ts(tb, TB)])
                nc.vector.tensor_add(OUTT[:, h, ts(tb, TB)], sw[:], raw[:])

            # ---- Phase A: projections (QK transposed + RoPE, V natural) ----
            for tb in range(NTB):
                xbs = []
                for ck in range(CK):
                    xb = xbp.tile([128, TB], MM_DT, tag="xtb", name=f"xb{tb}_{ck}")
                    nc.gpsimd.dma_start(xb[:], xT_v[:, ck, ts(tb, TB)])
                    xbs.append(xb)
                # bank assignment alternates per tb so sweeps pipeline
                a, b = (0, 2) if tb % 2 == 0 else (2, 0)
                pK = [pstile(f"P{a}", f"k{tb}a"), pstile(f"P{a+1}", f"k{tb}b")]
                pQ = [pstile(f"P{b}", f"q{tb}a"), pstile(f"P{b+1}", f"q{tb}b")]
                # K sweep
                for ck in range(CK):
                    for h in range(HL):
                        nc.tensor.matmul(
                            pK[h // 2][:, h % 2, :], wks[ck][:, ts(h, D)], xbs[ck][:],
                            start=(ck == 0), stop=(ck == CK - 1),
                        )
                for h in range(HL):
                    rope(pK[h // 2][:, h % 2, :], KT, h, tb)
                # Q sweep
                for ck in range(CK):
                    for h in range(HL):
                        nc.tensor.matmul(
                            pQ[h // 2][:, h % 2, :], wqs[ck][:, ts(h, D)], xbs[ck][:],
                            start=(ck == 0), stop=(ck == CK - 1),
                        )
                for h in range(HL):
                    rope(pQ[h // 2][:, h % 2, :], QT, h, tb)
                # V sweep (reuses K banks, freed by the K RoPE copies)
                pV = [pstile(f"P{a}", f"v{tb}a"), pstile(f"P{a+1}", f"v{tb}b")]
                for tco in range(NTB):
                    pv = pV[tco // 2][:, tco % 2, :]
                    for ck in range(CK):
                        nc.tensor.matmul(
                            pv, xbs[ck][:, ts(tco, 128)], wvt[:, ck, :],
                            start=(ck == 0), stop=(ck == CK - 1),
                        )
                    nc.scalar.copy(V[:, tb * NTB + tco, :], pv)

            # ---- Phase B: attention + interleaved output projection ----
            # The softmax denominator is computed on the PE (ones-matmuls
            # accumulating over the same restricted ranges as the A@V pass),
            # so no elementwise reduction tree is needed. Junk columns of et
            # (outside a chunk's allowed q-range) are never read; only the
            # partial diagonal subtiles need an explicit {0,1} mask multiply.
            for ib in range(NTB):
                ranges = [_q_range(ib, c) for c in range(NTC)]
                full = [c for c in range(NTC) if ranges[c] == (0, TB)]
                ff, fl = full[0], full[-1]
                order = [ff] + [c for c in range(NTC) if c not in (ff, fl)] + [fl]
                for h in range(HL):
                    ets = []
                    for cp in range(NTC // 2):
                        ps = pstile(f"P{cp % 2}", f"s{h}_{ib}_{cp}")
                        for k in range(2):
                            c = 2 * cp + k
                            lo, hi = ranges[c]
                            nc.tensor.matmul(
                                ps[:, k, ds(lo, hi - lo)],
                                KT[:, h, ts(c, 128)],
                                QT[:, h, ds(ib * TB + lo, hi - lo)],
                                start=True, stop=True,
                            )
                        et = etp.tile([128, 2, TB], MM_DT, tag="et")
                        # exp only the union of the two chunks' q-ranges:
                        # nothing ever reads et outside the per-chunk ranges
                        ulo = min(ranges[2 * cp][0], ranges[2 * cp + 1][0])
                        uhi = max(ranges[2 * cp][1], ranges[2 * cp + 1][1])
                        nc.scalar.activation(
                            et[:, :, ds(ulo, uhi - ulo)],
                            ps[:, :, ds(ulo, uhi - ulo)], AF.Exp, scale=scale,
                        )
                        # zero the (at most one) partial diagonal subtile
                        for k in range(2):
                            c = 2 * cp + k
                            for s in range(4 * ib, 4 * ib + 4):
                                if s - c in (0, 4):
                                    so = (s - 4 * ib) * 128
                                    mo = 0 if s - c == 0 else 128
                                    nc.vector.tensor_mul(
                                        et[:, k, ds(so, 128)],
                                        et[:, k, ds(so, 128)],
                                        maskb[:, ds(mo, 128)],
                                    )
                        ets.append(et)
                    # pre-sum pairs of full-range chunks on DVE so the PE
                    # denominator matmuls only cover the reduced set
                    fullpairs = [
                        cp for cp in range(NTC // 2)
                        if ranges[2 * cp] == (0, TB) and ranges[2 * cp + 1] == (0, TB)
                    ]
                    rest = [c for c in range(NTC) if c // 2 not in fullpairs]
                    us = []
                    for j, cp in enumerate(fullpairs):
                        u = zp.tile([128, TB], MM_DT, tag=f"u{j}", name=f"u{h}_{ib}_{j}")
                        nc.vector.tensor_add(u[:], ets[cp][:, 0, :], ets[cp][:, 1, :])
                        us.append(u)
                    qs = []
                    for j in range(0, len(us) - 1, 2):
                        nc.vector.tensor_add(us[j][:], us[j][:], us[j + 1][:])
                        qs.append(us[j])
                    if len(us) % 2:
                        qs.append(us[-1])
                    poz = pstile("P2", f"poz{h}_{ib}")
                    pso = poz[:, 0, :]
                    psz = poz[:, 1, :]
                    for idx, c in enumerate(order):
                        lo, hi = ranges[c]
                        nc.tensor.matmul(
                            pso[:, ds(lo, hi - lo)], V[:, c, ts(h, D)],
                            ets[c // 2][:, c % 2, ds(lo, hi - lo)],
                            start=(idx == 0), stop=(idx == NTC - 1),
                        )
                    # denominator: qs tiles (full width) + leftover chunks;
                    # first and last summands are full-width for start/stop
                    rest_f = [c for c in rest if ranges[c] == (0, TB)]
                    rest_p = [c for c in rest if ranges[c] != (0, TB)]
                    nz = len(qs) + len(rest) - 1
                    i = 0
                    nc.tensor.matmul(
                        psz, ones[:], qs[0][:], start=True, stop=(nz == 0)
                    )
                    for q in qs[1:]:
                        i += 1
                        nc.tensor.matmul(
                            psz, ones[:], q[:], start=False, stop=(i == nz)
                        )
                    for c in rest_p + rest_f:
                        i += 1
                        lo, hi = ranges[c]
                        nc.tensor.matmul(
                            psz[:, ds(lo, hi - lo)], ones[:],
                            ets[c // 2][:, c % 2, ds(lo, hi - lo)],
                            start=False, stop=(i == nz),
                        )
                    rz = smp.tile([128, TB], F32, tag="rz")
                    nc.vector.reciprocal_approx_fast(rz[:], psz)
                    nc.vector.tensor_mul(oT[:, h, ts(ib, TB)], pso, rz[:])
                # output projection for this i-block (all 4 heads done).
                # 1024-wide matmuls fill a whole 2-bank PSUM tile; alternate
                # tags P3/P2 so the Pool drain of one overlaps the next.
                g = 0
                for tto in range(NTB):
                    tt = ib * NTB + tto
                    for cbp in range(2):
                        po = pstile(f"P{3 - g % 2}", f"po{ib}_{tto}_{cbp}")
                        g += 1
                        for half in range(2):
                            for h in range(HL):
                                nc.tensor.matmul(
                                    po[:, half, :], oT[:, h, ts(tt, 128)],
                                    wof[:, h, ds((2 * cbp + half) * TB, TB)],
                                    start=(h == 0), stop=(h == HL - 1),
                                )
                        ob = ocb.tile([128, 2, TB], F32, tag="ob")
                        nc.vector.tensor_copy(ob[:], po[:])
                        nc.sync.dma_start(
                            out[ts(tt, 128), ds(cbp * 2 * TB, 2 * TB)], ob[:]
                        )

    nc.finalize()
    return nc


def _host_tables():
    inv_freq = (
        1.0 / (np.float32(ROPE_BASE) ** (np.arange(0, D, 2, dtype=np.float32) / np.float32(D)))
    ).astype(np.float32)
    t = np.arange(T, dtype=np.float32)
    freqs = (t[:, None] * inv_freq[None, :]).astype(np.float32)  # [T, 64]
    cos = np.cos(freqs).T.astype(np.float32)                     # [64, T]
    sin = np.sin(freqs).T.astype(np.float32)
    cosx = np.ascontiguousarray(np.concatenate([cos, cos], axis=0)).astype(NP_MM)
    sinx = np.ascontiguousarray(np.concatenate([-sin, sin], axis=0)).astype(NP_MM)
    p = np.arange(128, dtype=np.int64)[:, None]
    f = np.arange(128, dtype=np.int64)[None, :]
    blocks = []
    for delta_chunk in (0, 4):
        delta = 128 * delta_chunk + f - p          # i - j within the subtile
        blocks.append(~((delta >= 0) & (delta <= WINDOW - 1)))
    maskm = np.ascontiguousarray(np.concatenate(blocks, axis=1).astype(NP_MM))
    return cosx, sinx, maskm


def kernel(x, Wq, Wk, Wv, Wo):
    global _NC, LAST_RESULT
    if _NC is None:
        _NC = build_nc()
    x = np.asarray(x, dtype=np.float32)
    Wq = np.asarray(Wq, dtype=np.float32)
    Wk = np.asarray(Wk, dtype=np.float32)
    Wv = np.asarray(Wv, dtype=np.float32)
    Wo = np.asarray(Wo, dtype=np.float32)
    cosx, sinx, maskm = _host_tables()
    in_maps = []
    for core in range(NCORES):
        b, hg = divmod(core, NCORES // B)
        sl = slice(hg * HL * D, (hg + 1) * HL * D)
        in_maps.append(
            {
                "xT": np.ascontiguousarray(x[b].T.astype(NP_MM)),
                "wq": np.ascontiguousarray(Wq[:, sl].astype(NP_MM)),
                "wk": np.ascontiguousarray(Wk[:, sl].astype(NP_MM)),
                "wv": np.ascontiguousarray(Wv[:, sl].astype(NP_MM)),
                "wo": np.ascontiguousarray(Wo[sl, :].astype(NP_MM)),
                "cosx": cosx,
                "sinx": sinx,
                "maskm": maskm,
            }
        )
    res = run_bass_kernel_spmd(_NC, in_maps, list(range(NCORES)), trace=TRACE)
    LAST_RESULT = res
    out = np.zeros((B, T, C), dtype=np.float32)
    for core in range(NCORES):
        b = core // (NCORES // B)
        out[b] += res.results[core]["out"]
    return out


# revision 36
# speedup vs baseline: 1.1883x; 1.0275x over previous
"""Trainium2 Bass kernel for windowed (inverted-window) attention.

Problem: B=2, T=2048, C=2048, H=16 heads, D=128, WINDOW=512.
  q,k,v = x@Wq, x@Wk, x@Wv  (per-head reshape), RoPE on q,k,
  scores masked so positions INSIDE the causal window are masked out
  (attend only to j>i or j<i-511), softmax, o@Wo.

Sharding: 8 cores = 2 (batch) x 4 (head groups of 4 heads).
Each core computes its batch's 4 heads end-to-end plus a partial
output projection (row-chunk of Wo); host sums the 4 partials per batch.

Matmul operands are bf16 (fp32 PSUM accumulation); everything else fp32.

Optimizations over the first working version:
 - single PSUM pool (4 tags x 2 banks) spanning both phases: no pool-close
   barrier between projections and attention.
 - phase A grouped as K-sweep / Q-sweep / V-sweep per t-block with PSUM
   bank alternation, so RoPE drains overlap the next sweep's matmuls.
 - score/AV matmuls restricted to the contiguous allowed q-range per
   (i-block, k-chunk): fully-masked 128-wide subtiles are never computed.
   The mask is applied as min(et, {0,1e30}) which also zeroes stale
   columns, keeping the softmax-denominator tree exact.
 - RoPE arithmetic in bf16 (DVE 2x mode), exp on union ranges only.
 - PSUM->SBUF output copies on the (otherwise idle) Pool engine.
 - Wo kept resident in SBUF; mask/Wo loads issued during phase A.
"""

import sys
import numpy as np

for _p in ("/opt/trn_rl_repo",):
    if _p not in sys.path:
        sys.path.insert(0, _p)

import ml_dtypes  # noqa: E402

# If BASS_TRACE is set in the environment, run_bass_kernel_spmd imports
# antenv.axon_hooks, which this container does not ship. Register a stub
# so tracing degrades gracefully instead of crashing.
try:
    import antenv.axon_hooks  # noqa: F401
except ImportError:
    import types as _types

    _hooks = _types.ModuleType("antenv.axon_hooks")
    _hooks._hook = None
    _hooks.set_axon_ntff_profile_hook = lambda h: setattr(_hooks, "_hook", h)
    _hooks.get_axon_ntff_profile_hook = lambda: _hooks._hook
    sys.modules["antenv.axon_hooks"] = _hooks
    import antenv as _antenv

    _antenv.axon_hooks = _hooks
import concourse.bass as bass  # noqa: E402
import concourse.mybir as mybir  # noqa: E402
from concourse.bacc import Bacc  # noqa: E402
from concourse.tile import TileContext  # noqa: E402
from concourse.bass import ts, ds  # noqa: E402
from concourse.bass_utils import run_bass_kernel_spmd  # noqa: E402

B, T, C, H, D = 2, 2048, 2048, 16, 128
HL = 4                # heads per core
NCORES = 8
WINDOW = 512
ROPE_BASE = 10000.0
TB = 512              # i/t block size (matmul free dim)
NTB = T // TB         # 4
CK = C // 128         # 16 contraction chunks for projections
NTC = T // 128        # 16 j-chunks / t-chunks
MASK_W = 256          # two 128x128 partial-subtile masks (delta = 0 and 4)
F32 = mybir.dt.float32
BF16 = mybir.dt.bfloat16
AF = mybir.ActivationFunctionType

MM_DT = BF16          # dtype of every matmul operand tensor
NP_MM = ml_dtypes.bfloat16

_NC = None
TRACE = False
LAST_RESULT = None    # BassKernelResults of the most recent run (for test.py)


def _q_range(ib, c):
    """Contiguous allowed q-subchunk range for i-block ib vs key chunk c.

    Subtile (qsub s, key chunk c) is fully inside the masked window iff
    1 <= s - c <= 3 (then every i-j lies in [0, 511]).  Returns (lo, hi)
    in elements within the 512-wide i-block.
    """
    subs = [s for s in range(4 * ib, 4 * ib + 4) if not (1 <= s - c <= 3)]
    lo = (min(subs) - 4 * ib) * 128
    hi = (max(subs) - 4 * ib) * 128 + 128
    return lo, hi


def build_nc():
    nc = Bacc()
    # all inputs are repacked host-side to partition-major layouts so DMA
    # descriptors cover multi-KB contiguous runs per partition
    xT = nc.declare_dram_parameter("xT", [128, NTB, CK, TB], MM_DT, isOutput=False)
    wq = nc.declare_dram_parameter("wq", [128, CK, HL * D], MM_DT, isOutput=False)
    wk = nc.declare_dram_parameter("wk", [128, CK, HL * D], MM_DT, isOutput=False)
    wv = nc.declare_dram_parameter("wv", [128, CK, HL * D], MM_DT, isOutput=False)
    wo = nc.declare_dram_parameter("wo", [128, HL, C], MM_DT, isOutput=False)
    cosx = nc.declare_dram_parameter("cosx", [128, T], MM_DT, isOutput=False)
    sinx = nc.declare_dram_parameter("sinx", [128, T], MM_DT, isOutput=False)
    maskm = nc.declare_dram_parameter("maskm", [128, MASK_W], MM_DT, isOutput=False)
    out = nc.declare_dram_parameter("out", [T, C], F32, isOutput=True)

    scale = float(1.0 / np.sqrt(D))

    with TileContext(nc) as tc:
        with (
            tc.tile_pool(name="res", bufs=1) as res,      # long-lived residents
            tc.tile_pool(name="xbp", bufs=5) as xbp,      # streamed x quad-chunks
            tc.tile_pool(name="ropet", bufs=4) as ropet,
            tc.tile_pool(name="ropes", bufs=4) as ropes,
            tc.tile_pool(name="etp", bufs=10) as etp,
            tc.tile_pool(name="smp", bufs=2) as smp,
            tc.tile_pool(name="zp", bufs=1) as zp,
            tc.tile_pool(name="ocb", bufs=3) as ocb,
            tc.tile_pool(name="psum", bufs=1, space="PSUM") as psum,
        ):
            # ---- long-lived tensors; chunk-pair DMAs give 2KB descriptors ----
            wqt = res.tile([128, CK, HL * D], MM_DT, name="wqt")
            wkt = res.tile([128, CK, HL * D], MM_DT, name="wkt")
            for j in range(CK // 2):
                nc.sync.dma_start(wkt[:, 2 * j:2 * j + 2, :], wk[:, 2 * j:2 * j + 2, :])
                nc.sync.dma_start(wqt[:, 2 * j:2 * j + 2, :], wq[:, 2 * j:2 * j + 2, :])
            cosb = res.tile([128, T], MM_DT)
            nc.sync.dma_start(cosb[:], cosx[:])
            sinb = res.tile([128, T], MM_DT)
            nc.sync.dma_start(sinb[:], sinx[:])
            wvt = res.tile([128, CK, HL * D], MM_DT)
            nc.sync.dma_start(wvt[:], wv[:])
            maskb = res.tile([128, MASK_W], MM_DT)
            nc.sync.dma_start(maskb[:], maskm[:])
            wof = res.tile([128, HL, C], MM_DT)           # resident Wo
            nc.sync.dma_start(wof[:], wo[:])

            QT = res.tile([128, HL, T], MM_DT)    # q transposed [d, t]
            KT = res.tile([128, HL, T], MM_DT)
            V = res.tile([128, NTC, HL * D], MM_DT)   # v natural [t, hd]
            oT = res.tile([128, HL, T], MM_DT)    # per-head o transposed [d, t]
            ones = res.tile([128, 128], MM_DT)
            nc.vector.memset(ones[:], 1.0)

            # PSUM: 4 persistent double-bank tiles, alternated between phases
            def pstile(tag, nm):
                return psum.tile([128, 2, TB], F32, tag=tag, name=nm)

            def rope(ps_slice, OUTT, h, tb):
                # RoPE: out = raw*cos + swap(raw)*sin_signed, all bf16
                raw = ropet.tile([128, TB], MM_DT, tag="raw")
                nc.scalar.copy(raw[:], ps_slice)
                sw = ropes.tile([128, TB], MM_DT, tag="sw")
                # issue the half-swap DMAs from the scalar queue's DGE: the
                # sync queue streams weights and the SWDGE queue must stay
                # free for x prefetch (its FIFO is strictly in-order)
                nc.scalar.dma_start(sw[0:64, :], raw[64:128, :])
                nc.scalar.dma_start(sw[64:128, :], raw[0:64, :])
                nc.vector.tensor_mul(sw[:], sw[:], sinb[:, ts(tb, TB)])
                nc.vector.tensor_mul(raw[:], raw[:], cosb[:, ts(tb, TB)])
                nc.vector.tensor_add(OUTT[:, h, ts(tb, TB)], sw[:], raw[:])

            # ---- Phase A: projections (QK transposed + RoPE, V natural) ----
            for tb in range(NTB):
                xqs = []
                for q4 in range(CK // 4):
                    xq = xbp.tile([128, 4, TB], MM_DT, tag="xtb", name=f"xb{tb}_{q4}")
                    nc.gpsimd.dma_start(xq[:], xT[:, tb, 4 * q4:4 * q4 + 4, :])
                    xqs.append(xq)
                xbs = [xqs[ck // 4][:, ck % 4, :] for ck in range(CK)]
                # bank assignment alternates per tb so sweeps pipeline
                a, b = (0, 2) if tb % 2 == 0 else (2, 0)
                pK = [pstile(f"P{a}", f"k{tb}a"), pstile(f"P{a+1}", f"k{tb}b")]
                pQ = [pstile(f"P{b}", f"q{tb}a"), pstile(f"P{b+1}", f"q{tb}b")]
                # K sweep
                for ck in range(CK):
                    for h in range(HL):
                        nc.tensor.matmul(
                            pK[h // 2][:, h % 2, :], wkt[:, ck, ts(h, D)], xbs[ck],
                            start=(ck == 0), stop=(ck == CK - 1),
                        )
                for h in range(HL):
                    rope(pK[h // 2][:, h % 2, :], KT, h, tb)
                # Q sweep
                for ck in range(CK):
                    for h in range(HL):
                        nc.tensor.matmul(
                            pQ[h // 2][:, h % 2, :], wqt[:, ck, ts(h, D)], xbs[ck],
                            start=(ck == 0), stop=(ck == CK - 1),
                        )
                for h in range(HL):
                    rope(pQ[h // 2][:, h % 2, :], QT, h, tb)
                # V sweep (reuses K banks, freed by the K RoPE copies)
                pV = [pstile(f"P{a}", f"v{tb}a"), pstile(f"P{a+1}", f"v{tb}b")]
                for tco in range(NTB):
                    pv = pV[tco // 2][:, tco % 2, :]
                    for ck in range(CK):
                        nc.tensor.matmul(
                            pv, xqs[ck // 4][:, ck % 4, ts(tco, 128)], wvt[:, ck, :],
                            start=(ck == 0), stop=(ck == CK - 1),
                        )
                    nc.scalar.copy(V[:, tb * NTB + tco, :], pv)

            # ---- Phase B: attention + interleaved output projection ----
            # The softmax denominator is computed on the PE (ones-matmuls
            # accumulating over the same restricted ranges as the A@V pass),
            # so no elementwise reduction tree is needed. Junk columns of et
            # (outside a chunk's allowed q-range) are never read; only the
            # partial diagonal subtiles need an explicit {0,1} mask multiply.
            def emit_oproj(ib):
                # 1024-wide drains fill a whole 2-bank PSUM tile; alternate
                # tags P3/P2 so the DVE drain of one overlaps the next
                g = 0
                for tto in range(NTB):
                    tt = ib * NTB + tto
                    for cbp in range(2):
                        po = pstile(f"P{3 - g % 2}", f"po{ib}_{tto}_{cbp}")
                        g += 1
                        for half in range(2):
                            for h in range(HL):
                                nc.tensor.matmul(
                                    po[:, half, :], oT[:, h, ts(tt, 128)],
                                    wof[:, h, ds((2 * cbp + half) * TB, TB)],
                                    start=(h == 0), stop=(h == HL - 1),
                                )
                        ob = ocb.tile([128, 2, TB], F32, tag="ob")
                        nc.vector.tensor_copy(ob[:], po[:])
                        nc.sync.dma_start(
                            out[ts(tt, 128), ds(cbp * 2 * TB, 2 * TB)], ob[:]
                        )

            for ib in range(NTB):
                ranges = [_q_range(ib, c) for c in range(NTC)]
                full = [c for c in range(NTC) if ranges[c] == (0, TB)]
                ff, fl = full[0], full[-1]
                order = [ff] + [c for c in range(NTC) if c not in (ff, fl)] + [fl]
                for h in range(HL):
                    ets = []
                    for cp in range(NTC // 2):
                        ps = pstile(f"P{cp % 2}", f"s{h}_{ib}_{cp}")
                        for k in range(2):
                            c = 2 * cp + k
                            lo, hi = ranges[c]
                            nc.tensor.matmul(
                                ps[:, k, ds(lo, hi - lo)],
                                KT[:, h, ts(c, 128)],
                                QT[:, h, ds(ib * TB + lo, hi - lo)],
                                start=True, stop=True,
                            )
                        et = etp.tile([128, 2, TB], MM_DT, tag="et")
                        # exp only the union of the two chunks' q-ranges:
                        # nothing ever reads et outside the per-chunk ranges
                        ulo = min(ranges[2 * cp][0], ranges[2 * cp + 1][0])
                        uhi = max(ranges[2 * cp][1], ranges[2 * cp + 1][1])
                        nc.scalar.activation(
                            et[:, :, ds(ulo, uhi - ulo)],
                            ps[:, :, ds(ulo, uhi - ulo)], AF.Exp, scale=scale,
                        )
                        # zero the (at most one) partial diagonal subtile
                        for k in range(2):
                            c = 2 * cp + k
                            for s in range(4 * ib, 4 * ib + 4):
                                if s - c in (0, 4):
                                    so = (s - 4 * ib) * 128
                                    mo = 0 if s - c == 0 else 128
                                    nc.vector.tensor_mul(
                                        et[:, k, ds(so, 128)],
                                        et[:, k, ds(so, 128)],
                                        maskb[:, ds(mo, 128)],
                                    )
                        ets.append(et)
                    # pre-sum pairs of full-range chunks on DVE so the PE
                    # denominator matmuls only cover the reduced set
                    fullpairs = [
                        cp for cp in range(NTC // 2)
                        if ranges[2 * cp] == (0, TB) and ranges[2 * cp + 1] == (0, TB)
                    ]
                    rest = [c for c in range(NTC) if c // 2 not in fullpairs]
                    us = []
                    for j, cp in enumerate(fullpairs):
                        u = zp.tile([128, TB], MM_DT, tag=f"u{j}", name=f"u{h}_{ib}_{j}")
                        nc.vector.tensor_add(u[:], ets[cp][:, 0, :], ets[cp][:, 1, :])
                        us.append(u)
                    qs = []
                    for j in range(0, len(us) - 1, 2):
                        nc.vector.tensor_add(us[j][:], us[j][:], us[j + 1][:])
                        qs.append(us[j])
                    if len(us) % 2:
                        qs.append(us[-1])
                    poz = pstile("P2", f"poz{h}_{ib}")
                    pso = poz[:, 0, :]
                    psz = poz[:, 1, :]
                    for idx, c in enumerate(order):
                        lo, hi = ranges[c]
                        nc.tensor.matmul(
                            pso[:, ds(lo, hi - lo)], V[:, c, ts(h, D)],
                            ets[c // 2][:, c % 2, ds(lo, hi - lo)],
                            start=(idx == 0), stop=(idx == NTC - 1),
                        )
                    # denominator: qs tiles (full width) + leftover chunks;
                    # first and last summands are full-width for start/stop
                    rest_f = [c for c in rest if ranges[c] == (0, TB)]
                    rest_p = [c for c in rest if ranges[c] != (0, TB)]
                    nz = len(qs) + len(rest) - 1
                    i = 0
                    nc.tensor.matmul(
                        psz, ones[:], qs[0][:], start=True, stop=(nz == 0)
                    )
                    for q in qs[1:]:
                        i += 1
                        nc.tensor.matmul(
                            psz, ones[:], q[:], start=False, stop=(i == nz)
                        )
                    for c in rest_p + rest_f:
                        i += 1
                        lo, hi = ranges[c]
                        nc.tensor.matmul(
                            psz[:, ds(lo, hi - lo)], ones[:],
                            ets[c // 2][:, c % 2, ds(lo, hi - lo)],
                            start=False, stop=(i == nz),
                        )
                    rz = smp.tile([128, TB], F32, tag="rz")
                    nc.vector.reciprocal_approx_fast(rz[:], psz)
                    nc.vector.tensor_mul(oT[:, h, ts(ib, TB)], pso, rz[:])
                    # emit the previous block's output projection only after
                    # this block's first head, so the exp pipeline never
                    # drains across the i-block boundary
                    if h == 0 and ib > 0:
                        emit_oproj(ib - 1)
            emit_oproj(NTB - 1)

    nc.finalize()
    return nc


def _host_tables():
    inv_freq = (
        1.0 / (np.float32(ROPE_BASE) ** (np.arange(0, D, 2, dtype=np.float32) / np.float32(D)))
    ).astype(np.float32)
    t = np.arange(T, dtype=np.float32)
    freqs = (t[:, None] * inv_freq[None, :]).astype(np.float32)  # [T, 64]
    cos = np.cos(freqs).T.astype(np.float32)                     # [64, T]
    sin = np.sin(freqs).T.astype(np.float32)
    cosx = np.ascontiguousarray(np.concatenate([cos, cos], axis=0)).astype(NP_MM)
    sinx = np.ascontiguousarray(np.concatenate([-sin, sin], axis=0)).astype(NP_MM)
    p = np.arange(128, dtype=np.int64)[:, None]
    f = np.arange(128, dtype=np.int64)[None, :]
    blocks = []
    for delta_chunk in (0, 4):
        delta = 128 * delta_chunk + f - p          # i - j within the subtile
        blocks.append(~((delta >= 0) & (delta <= WINDOW - 1)))
    maskm = np.ascontiguousarray(np.concatenate(blocks, axis=1).astype(NP_MM))
    return cosx, sinx, maskm


def kernel(x, Wq, Wk, Wv, Wo):
    global _NC, LAST_RESULT
    if _NC is None:
        _NC = build_nc()
    x = np.asarray(x, dtype=np.float32)
    Wq = np.asarray(Wq, dtype=np.float32)
    Wk = np.asarray(Wk, dtype=np.float32)
    Wv = np.asarray(Wv, dtype=np.float32)
    Wo = np.asarray(Wo, dtype=np.float32)
    cosx, sinx, maskm = _host_tables()
    in_maps = []
    def _pmajor_cols(w):       # [C, F] -> [128, C//128, F]
        return np.ascontiguousarray(
            w.reshape(w.shape[0] // 128, 128, w.shape[1]).transpose(1, 0, 2)
            .astype(NP_MM)
        )

    xTs = {}
    for b in range(B):
        xt = x[b].T                                     # [C, T]
        xTs[b] = np.ascontiguousarray(
            xt.reshape(CK, 128, NTB, TB).transpose(1, 2, 0, 3).astype(NP_MM)
        )                                               # [128, NTB, CK, TB]
    for core in range(NCORES):
        b, hg = divmod(core, NCORES // B)
        sl = slice(hg * HL * D, (hg + 1) * HL * D)
        in_maps.append(
            {
                "xT": xTs[b],
                "wq": _pmajor_cols(Wq[:, sl]),
                "wk": _pmajor_cols(Wk[:, sl]),
                "wv": _pmajor_cols(Wv[:, sl]),
                "wo": _pmajor_cols(Wo[sl, :]),
                "cosx": cosx,
                "sinx": sinx,
                "maskm": maskm,
            }
        )
    res = run_bass_kernel_spmd(_NC, in_maps, list(range(NCORES)), trace=TRACE)
    LAST_RESULT = res
    out = np.zeros((B, T, C), dtype=np.float32)
    for core in range(NCORES):
        b = core // (NCORES // B)
        out[b] += res.results[core]["out"]
    return out
